# revision 62
# baseline (speedup 1.0000x reference)
"""Causal self-attention on 8 Trainium2 NeuronCores (Bass/Tile).

Sharding: core c -> (batch b = c//2, head-group g = c%2).  Each core runs
attention for 8 heads of one batch element: qkv projection (columns of
w_qkv for its heads), causal softmax attention, and its half of the
output projection (rows of w_proj).  Host sums the two partial
projections per batch and transposes back.

All on-device tensors are bf16 (cast on host; PSUM accumulation stays
fp32) - same PE rate as fp32r but half the DMA/SBUF and 2x DVE.

Dataflow is fully transposed on-device (the contraction dim always sits
on SBUF partitions, so no on-device transposes are needed anywhere):
  xT [D, T] -> qT/kT [64, T] per head -> S^T [kidx, q] blocks -> exp ->
  PV gives out^T [d, q]; an appended ones-column in the stationary
  operand makes the PE produce the softmax denominators for free ->
  w_proj consumed natively as lhsT -> oT [D, T] partial output.
Softmax skips max-subtraction (scores ~ N(0,1); exp is safe) and
causal-skips fully-masked blocks in scores, exp, and PV.

Schedule (single Tile program; engines overlap by data deps):
  1. K/Q projections for head-pair 0 run dt-outer (8 PSUM banks of
     interleaved accumulation) so the PE starts as soon as the first
     xT chunk lands instead of after the whole input DMA.
  2. Attention runs as two q-half sweeps over all heads (h0 = q<1024,
     then h1).  The PV accumulation trails scores/exp by 2 kt steps in
     a pipeline carried ACROSS head boundaries, and a "fill queue" of
     PE work - K/Q projection passlets for pairs 1-3 (two PSUM banks,
     one dt step per fill step) and V-projection tiles - is consumed
     between score steps.  This keeps the PE ~98% busy through h0
     while the scalar engine's exp stream (the true attention-phase
     bottleneck: exp has no 16-bit speedup and runs only on ACT)
     catches up; h1 interleaves the q<1024 output-projection groups
     the same way.
  3. Softmax denominators: reciprocal runs directly on the PSUM sums
     row into a [1, 512] SBUF row, which is partition-broadcast via a
     DRAM bounce (SBUF DMA cannot replicate partitions).  Each q-chunk's
     chain is emitted as soon as its last PV lands, so it hides under
     later kt steps; the final multiply is deferred a few pipeline pops
     so its DMA dependency cannot head-of-line-block the DVE FIFO
     (which also carries the latency-critical causal-mask muls).
  4. Output projection drains through PSUM->SBUF copies on DVE/ACT,
     DMAing bf16 partials; host upcasts and sums.
  (GPSIMD/Pool cannot touch PSUM, matmul outputs cannot cross a 2KB
  PSUM bank, and only ACT can exp - these constraints shape the whole
  schedule.)

V layout per kidx-tile and head pair: [V_even | J | V_odd] where J is 64
cols of zeros with a 1.0 in col 32.  PV stationary for an even head is
[V_h | J] (out rows 0-63, sums row 96), for an odd head [J | V_h] (sums
row 32, out rows 64-127) -- keeping every later elementwise op
partition-aligned and 32-aligned for the DVE.
"""

import sys

sys.path.insert(0, "/opt/trn_rl_repo")

import numpy as np

N_CORES = 8
B, T, D = 4, 2048, 1024
H, HD = 16, 64
HG = 2  # head groups (tensor parallel)
HC = H // HG  # heads per core
CD = HC * HD  # per-core qkv width (512)

_CACHE = {}


def build_attention_kernel(T, D, HC, HD, n_cores=N_CORES, loop_n=1):
    """Build + compile the per-core Bass module (same module on all cores).

    loop_n > 1 wraps the whole body in a hardware For_i loop (for wall-clock
    HW-time calibration; the kernel is idempotent so outputs stay valid).
    """
    from contextlib import ExitStack

    import concourse.bass as bass
    import concourse.mybir as mybir
    import concourse.tile as tile
    from concourse import bacc
    from concourse.masks import make_upper_triangular

    f32 = mybir.dt.float32
    bf16 = mybir.dt.bfloat16
    EXP = mybir.ActivationFunctionType.Exp

    CD = HC * HD
    DT = D // 128  # contraction chunks over D
    NT = T // 128  # T tiles / kidx tiles
    CT = CD // 128  # col tiles of q/k block (= head pairs)
    HW = T // 2  # q-half width
    NQC = HW // 512  # 512-wide q chunks per half
    PB = 3 * HD  # V_sb block width per head pair: [V_even | J | V_odd]
    VW = CT * PB  # V_sb row width
    scale = 1.0 / float(np.sqrt(HD))

    nc = bacc.Bacc("TRN2", target_bir_lowering=False, debug=False, num_devices=n_cores)

    xT_d = nc.dram_tensor("xT", [D, T], bf16, kind="ExternalInput").ap()
    wq_d = nc.dram_tensor("wq", [D, CD], bf16, kind="ExternalInput").ap()
    wk_d = nc.dram_tensor("wk", [D, CD], bf16, kind="ExternalInput").ap()
    wv_d = nc.dram_tensor("wv", [D, CD], bf16, kind="ExternalInput").ap()
    wp_d = nc.dram_tensor("wp", [CD, D], bf16, kind="ExternalInput").ap()
    jc_d = nc.dram_tensor("jc", [128, HD], bf16, kind="ExternalInput").ap()
    oT_d = nc.dram_tensor("oT", [D, T], bf16, kind="ExternalOutput").ap()

    with tile.TileContext(nc) as tc, ExitStack() as ctx:
        if loop_n > 1:
            ctx.enter_context(tc.For_i(0, loop_n))
        # ---- persistent SBUF ----
        pers = ctx.enter_context(tc.tile_pool(name="pers", bufs=1))
        V_sb = pers.tile([128, NT, VW], bf16)
        QT_sb = pers.tile([128, CT, T], bf16)
        KT_sb = pers.tile([128, CT, T], bf16)
        oT_sb = pers.tile([128, CT, T], bf16)
        utri = pers.tile([128, 128], bf16)
        make_upper_triangular(nc, utri[:], val=1.0, diag=True)

        def pv_lhsT(kt, h, hh):
            # even head: [V_h | J] (block cols 0:128); odd: [J | V_h] (64:192)
            base = (h // 2) * PB + (HD if hh else 0)
            return V_sb[:, kt, base : base + 2 * HD]

        # ---- weight + xT DMAs, chunked per dt so the PE starts after the
        # first ~1MB instead of the whole input set ----
        pw = ctx.enter_context(tc.tile_pool(name="pw", bufs=1))
        xT_sb = pw.tile([128, DT, T], bf16)
        wk_sb = pw.tile([128, DT, CD], bf16)
        wq_sb = pw.tile([128, DT, CD], bf16)
        wv_sb = pw.tile([128, DT, CD], bf16)
        for dt in range(DT):
            sl = slice(dt * 128, (dt + 1) * 128)
            nc.sync.dma_start(out=wk_sb[:, dt, :], in_=wk_d[sl, :])
            if dt == 0:  # halved first chunk: PE starts after 384KB, not 640
                nc.sync.dma_start(out=xT_sb[:, 0, : T // 2], in_=xT_d[sl, : T // 2])
                nc.sync.dma_start(out=xT_sb[:, 0, T // 2 :], in_=xT_d[sl, T // 2 :])
            else:
                nc.sync.dma_start(out=xT_sb[:, dt, :], in_=xT_d[sl, :])
            nc.sync.dma_start(out=wq_sb[:, dt, :], in_=wq_d[sl, :])
        for dt in range(DT):
            nc.sync.dma_start(out=wv_sb[:, dt, :], in_=wv_d[dt * 128 : (dt + 1) * 128, :])
        wp_sb = pw.tile([128, CT, D], bf16)
        nc.sync.dma_start(out=wp_sb[:], in_=wp_d.rearrange("(a p) c -> p a c", p=128))

        J_sb = pers.tile([128, HD], bf16)
        nc.sync.dma_start(out=J_sb[:], in_=jc_d[:])
        for kt in range(NT):
            for p_i in range(CT):
                nc.gpsimd.tensor_copy(
                    V_sb[:, kt, p_i * PB + HD : p_i * PB + 2 * HD],
                    J_sb[:],
                )

        # ---- K/Q projections for pair 0 only, dt-outer (the PE streams
        # behind the xT DMA).  Pairs 1-3 are projected later as "passlet"
        # fill steps interleaved into the attention sweeps, where the PE
        # would otherwise idle waiting on the scalar engine's exps.
        with tc.tile_pool(name="kqps", bufs=8, space="PSUM") as kqps:
            for w_sb, T_sb in ((wk_sb, KT_sb), (wq_sb, QT_sb)):
                ps = [
                    kqps.tile([128, 512], f32, tag="kq", name=f"kq0_{j}")
                    for j in range(4)
                ]
                for dt in range(DT):
                    for qc in range(4):
                        nc.tensor.matmul(
                            ps[qc][:],
                            w_sb[:, dt, 0:128],
                            xT_sb[:, dt, qc * 512 : (qc + 1) * 512],
                            start=(dt == 0),
                            stop=(dt == DT - 1),
                        )
                for qc in range(4):
                    # drain alternately on DVE and the (still idle) ACT
                    if qc % 2:
                        nc.scalar.copy(T_sb[:, 0, qc * 512 : (qc + 1) * 512], ps[qc][:])
                    else:
                        nc.vector.tensor_copy(T_sb[:, 0, qc * 512 : (qc + 1) * 512], ps[qc][:])

        # ---- attention: q-window sweeps over all heads, V + out-proj
        # interleaved into the PE stream ----
        # h0 runs as two 512-wide windows (1 PSUM acc bank per head, so 4
        # normalize chains ride in flight); h1 as one 1024-wide window.
        # After a q-range is complete for all heads, its output-projection
        # groups are interleaved one per head-half of later windows.
        with (
            tc.tile_pool(name="pexp", bufs=8) as pexp,
            tc.tile_pool(name="pnrm", bufs=4) as pnrm,
            tc.tile_pool(name="ppo", bufs=4) as ppo,
            tc.tile_pool(name="pdr", bufs=4, space="DRAM") as pdr,
            tc.tile_pool(name="sps", bufs=2, space="PSUM") as sps,
            tc.tile_pool(name="aps", bufs=4, space="PSUM") as aps,
        ):
            v_next = [0]  # next V tile to project (global across windows)

            def v_tile():
                # V[t, vcol] = x @ wv for one 128-token tile, then pack
                # [V_even | J | V_odd] per head pair (J written at init)
                t = v_next[0]
                v_next[0] += 1
                v_ps = sps.tile([128, HW], f32, tag="sps", name=f"vps{t}")
                for dt in range(DT):
                    nc.tensor.matmul(
                        v_ps[:, :CD],
                        xT_sb[:, dt, t * 128 : (t + 1) * 128],
                        wv_sb[:, dt, :],
                        start=(dt == 0),
                        stop=(dt == DT - 1),
                    )
                Vv = V_sb.rearrange("p t (c w) -> p t c w", w=PB)
                pv = v_ps[:, :CD].rearrange("p (c two h) -> p c two h", two=2, h=HD)
                nc.vector.tensor_copy(Vv[:, t, :, 0:HD], pv[:, :, 0, :])
                nc.vector.tensor_copy(Vv[:, t, :, 2 * HD : PB], pv[:, :, 1, :])

            oproj = [0]  # next out-proj group (qc, nt); emitted once the
            # qc's q-range is complete for all heads

            def oproj_group(done_q):
                qc, nt = oproj[0] // (D // 128), oproj[0] % (D // 128)
                if (qc + 1) * 512 > done_q:
                    return
                oproj[0] += 1
                p_ps = sps.tile([128, 512], f32, tag="sps", name=f"pps{qc}_{nt}")
                for ct in range(CT):
                    nc.tensor.matmul(
                        p_ps[:],
                        wp_sb[:, ct, nt * 128 : (nt + 1) * 128],
                        oT_sb[:, ct, qc * 512 : (qc + 1) * 512],
                        start=(ct == 0),
                        stop=(ct == CT - 1),
                    )
                po = ppo.tile([128, 512], bf16, tag="po")
                if nt % 2 and oproj[0] > 16:  # tail groups: ACT is idle there
                    nc.scalar.copy(po[:], p_ps[:])
                else:
                    nc.vector.tensor_copy(po[:], p_ps[:])
                nc.sync.dma_start(
                    out=oT_d[nt * 128 : (nt + 1) * 128, qc * 512 : (qc + 1) * 512],
                    in_=po[:],
                )

            # PV trails S/exp by 2 kt steps GLOBALLY - the pipeline carries
            # across head-half boundaries, so one head's tail exps overlap
            # the next head's scores instead of draining the PE.
            pend = []
            # DVE executes in order, so the normalize mul (which waits ~4us
            # on the reciprocal's DRAM-bounce broadcast) is emitted a few
            # pipeline pops late - by then its DMA has landed and it cannot
            # head-of-line-block the latency-critical diag-mask muls.
            popc = [0]
            deferred = []  # (due_popc, thunk)

            def flush_deferred(all=False):
                while deferred and (all or deferred[0][0] <= popc[0]):
                    deferred.pop(0)[1]()

            def make_head(q0, qw, p_i, hh):
                h = 2 * p_i + hh
                nqc = qw // 512
                # ones sits at J col HD//2: sums row = HD + HD//2 (even
                # head, lhsT=[V_h|J]) or HD//2 (odd head)
                sum_row = HD // 2 if hh else HD + HD // 2
                out_lo = HD if hh else 0
                accs = [
                    aps.tile([128, 512], f32, name=f"acc{q0}_{h}_{i}", tag="acc")
                    for i in range(nqc)
                ]

                def norm_qcl(qcl):
                    # normalize chunk: out^T[d,q] * (1/sums[q]).  reciprocal
                    # reads the PSUM sums row directly; a DRAM bounce
                    # broadcasts it across partitions (only DMA can).
                    # Emitted as soon as the chunk's last PV lands so the
                    # chain hides under later kts; the final mul is deferred.
                    rrow = pnrm.tile([1, 512], f32, name=f"rr{q0}_{h}_{qcl}", tag="rr")
                    nc.vector.reciprocal(rrow[:], accs[qcl][sum_row : sum_row + 1, :])
                    scr = pdr.tile([1, 512], f32, name=f"scr{q0}_{h}_{qcl}", tag="scr")
                    nc.sync.dma_start(out=scr[:], in_=rrow[:])
                    rb = pnrm.tile([128, 512], f32, name=f"rb{q0}_{h}_{qcl}", tag="rb")
                    sbc = bass.AP(tensor=scr.tensor, offset=scr.offset, ap=[[0, HD], [1, 512]])
                    nc.sync.dma_start(out=rb[out_lo : out_lo + HD, :], in_=sbc)

                    def mul():
                        nc.vector.tensor_mul(
                            oT_sb[out_lo : out_lo + HD, p_i, q0 + qcl * 512 : q0 + (qcl + 1) * 512],
                            accs[qcl][out_lo : out_lo + HD, :],
                            rb[out_lo : out_lo + HD, :],
                        )

                    if q0 == HW and h >= 2 * CT - 2:
                        mul()  # last heads: the out-proj tail waits on these
                    else:
                        deferred.append((popc[0] + 4, mul))

                def emit_pv(kt, ex, lo):
                    for qcl in range(nqc):
                        rlo = max(lo - qcl * 512, 0)
                        if rlo >= 512:
                            continue  # block fully above this q chunk
                        last_kt = (q0 + (qcl + 1) * 512) // 128 - 1
                        nc.tensor.matmul(
                            accs[qcl][:, rlo:],
                            pv_lhsT(kt, h, hh),
                            ex[:, qcl * 512 + rlo : (qcl + 1) * 512],
                            start=(kt == 0),
                            stop=(kt == last_kt),
                        )
                        if kt == last_kt:
                            norm_qcl(qcl)

                return emit_pv

            def try_pops(limit=99):
                # pend entries are (emit_pv, kt, ex, lo); entry kt's PV
                # consumes V tile kt, so pops gate on the V counter
                n = 0
                while len(pend) > 2 and pend[0][1] < v_next[0] and n < limit:
                    e = pend.pop(0)
                    e[0](*e[1:])
                    popc[0] += 1
                    flush_deferred()
                    n += 1

            # K/Q projections for pairs 1-3 as generator "passlets": two
            # 512-wide PSUM accumulations (from the attention acc pool) that
            # advance one dt step per fill-step.  Consumed between score
            # steps, so the scalar engine's exp stream hides them.
            def passlet(w_sb, T_sb, pp, qcp):
                ps = [
                    aps.tile([128, 512], f32, tag="acc", name=f"kqf{pp}_{qc}")
                    for qc in qcp
                ]
                for dt in range(DT):
                    for i, qc in enumerate(qcp):
                        nc.tensor.matmul(
                            ps[i][:],
                            w_sb[:, dt, pp * 128 : (pp + 1) * 128],
                            xT_sb[:, dt, qc * 512 : (qc + 1) * 512],
                            start=(dt == 0),
                            stop=(dt == DT - 1),
                        )
                    yield
                for i, qc in enumerate(qcp):
                    if qc % 2:
                        nc.scalar.copy(T_sb[:, pp, qc * 512 : (qc + 1) * 512], ps[i][:])
                    else:
                        nc.vector.tensor_copy(T_sb[:, pp, qc * 512 : (qc + 1) * 512], ps[i][:])

            def v_gen():
                v_tile()
                yield

            # fill queue: (needed_by_pair, generator).  V tiles t=8..15 (only
            # consumed by the h1 sweep) are spread among the K/Q passlets.
            fill = []
            for pp in range(1, CT):
                for w_sb, T_sb in ((wk_sb, KT_sb), (wq_sb, QT_sb)):
                    fill.append((pp, passlet(w_sb, T_sb, pp, (0, 1))))
                    fill.append((pp, passlet(w_sb, T_sb, pp, (2, 3))))
            for i in range(NT - HW // 128, 0, -1):
                fill.insert(i * 5 // 2, (99, v_gen()))

            def fill_step(n):
                while n > 0 and fill:
                    try:
                        next(fill[0][1])
                    except StopIteration:
                        fill.pop(0)
                        continue
                    n -= 1

            def fill_drain(pp):
                # scan (not prefix-drain): passlets can sit behind V entries
                for e in [e for e in fill if e[0] <= pp]:
                    for _ in e[1]:
                        pass
                    fill.remove(e)

            for q0, qw in ((0, HW), (HW, HW)):
                kt_hi = (q0 + qw) // 128
                for p_i in range(CT):
                    if q0 == HW:
                        fill_drain(99)  # h1 needs all V tiles projected
                    else:
                        fill_drain(p_i)  # scores below need pair p_i's K/Q
                    for hh in range(2):
                        h = 2 * p_i + hh
                        QTh = QT_sb[hh * 64 : (hh + 1) * 64, p_i, :]
                        KTh = KT_sb[hh * 64 : (hh + 1) * 64, p_i, :]
                        emit_pv = make_head(q0, qw, p_i, hh)
                        for kt in range(kt_hi):
                            # the first head of sweep h0 projects the V tiles
                            # its own PVs consume (t = kt just in time)
                            if q0 == 0 and h == 0 and v_next[0] < kt_hi:
                                v_tile()
                            lo = max(kt * 128 - q0, 0)
                            s_ps = sps.tile([128, qw], f32, tag="sps", name=f"sps{q0}_{h}_{kt}")
                            c = lo
                            while c < qw:
                                c1 = min((c // 512 + 1) * 512, qw)
                                nc.tensor.matmul(
                                    s_ps[:, c:c1],
                                    KTh[:, kt * 128 : (kt + 1) * 128],
                                    QTh[:, q0 + c : q0 + c1],
                                    start=True,
                                    stop=True,
                                )
                                c = c1
                            ex = pexp.tile([128, qw], bf16, tag="ex", name=f"ex{q0}_{h}_{kt}")
                            nc.scalar.activation(ex[:, lo:], s_ps[:, lo:], EXP, scale=scale)
                            if kt * 128 >= q0:  # diagonal block in this half
                                dl = kt * 128 - q0
                                nc.vector.tensor_mul(ex[:, dl : dl + 128], ex[:, dl : dl + 128], utri[:])
                            pend.append((emit_pv, kt, ex, lo))
                            # PE filler between score steps: K/Q passlet dt
                            # steps / V tiles ride where the PE would wait
                            # for the exp stream
                            fill_step(1 if (q0 == 0 and h == 0) else 2)
                            try_pops(limit=2)
                        # out-proj filler for q < 1024 rides the h1 sweep
                        oproj_group(q0)
                        oproj_group(q0)

            while pend:
                e = pend.pop(0)
                e[0](*e[1:])
                popc[0] += 1
            flush_deferred(all=True)
            # remaining out-proj groups (q >= 1024)
            while oproj[0] < (T // 512) * (D // 128):
                oproj_group(T)

    nc.compile()
    return nc


def _get_compiled():
    key = (T, D, HC, HD)
    if key not in _CACHE:
        _CACHE[key] = build_attention_kernel(*key)
    return _CACHE[key]


def shard_inputs(x, w_qkv, w_proj):
    import ml_dtypes

    bf = ml_dtypes.bfloat16
    jc = np.zeros((128, HD), bf)
    jc[:, HD // 2] = 1.0
    xb = x.astype(bf)
    wqkv = w_qkv.astype(bf)
    wp = w_proj.astype(bf)
    in_maps = []
    for c in range(N_CORES):
        b, g = c // HG, c % HG
        in_maps.append(
            dict(
                jc=jc,
                xT=np.ascontiguousarray(xb[b].T),
                wq=np.ascontiguousarray(wqkv[:, g * CD : (g + 1) * CD]),
                wk=np.ascontiguousarray(wqkv[:, D + g * CD : D + (g + 1) * CD]),
                wv=np.ascontiguousarray(wqkv[:, 2 * D + g * CD : 2 * D + (g + 1) * CD]),
                wp=np.ascontiguousarray(wp[g * CD : (g + 1) * CD, :]),
            )
        )
    return in_maps


def gather_outputs(results):
    out = np.empty((B, T, D), np.float32)
    for b in range(B):
        acc = results[HG * b]["oT"].astype(np.float32) + results[HG * b + 1]["oT"].astype(np.float32)
        out[b] = acc.T
    return out


def kernel(x, w_qkv, w_proj):
    from concourse.bass_utils import run_bass_kernel_spmd

    x = np.asarray(x, dtype=np.float32)
    w_qkv = np.asarray(w_qkv, dtype=np.float32)
    w_proj = np.asarray(w_proj, dtype=np.float32)
    nc = _get_compiled()
    res = run_bass_kernel_spmd(nc, shard_inputs(x, w_qkv, w_proj), list(range(N_CORES)))
    return gather_outputs(res.results)
